# revision 34
# baseline (speedup 1.0000x reference)
"""Trainium2 Bass kernel for blocked-DCT high-frequency extractor.

Computes, for x (64, 3, 512, 512) f32:
  gray = 0.299*R + 0.587*G + 0.114*B                     (B,1,H,W)
  per 8x8 block:  Y = mask * (D @ block @ D.T)           (2D DCT + high-pass)
  output (64, 1, 512, 512) f32

Strategy: pure data parallel over batch (8 batches/core on 8 cores). The
kernel is HBM-bound: 24 MiB in + 8 MiB out per core => ~94 us floor at the
~358 GB/s per-core HBM limit. All compute engines are kept below the DMA
stream rate (~2.74 us per 128-row chunk) and the per-chunk work is
SOFTWARE-PIPELINED across three stages so the in-order engine streams never
serialize on the cross-engine dependency chain:

  stage A (chunk k):   in-DMA -> gray (DVE stt / ACT mul / GpSimd add,
                       rounded to f32r) -> H-DCT matmul (f32r, 1 cyc/row)
  stage B (chunk k-1): ACT PSUM->SBUF cast to bf16 -> DVE 32x32 block
                       transpose (bf16) -> W-DCT matmul (bf16)
  stage C (chunk k-2): high-pass mask on ACT (two strided PSUM->SBUF
                       copies; u<4 columns scaled by a per-partition 0/1
                       vector) -> DVE block transpose back (f32)
  stage D (chunk k-3): 256 KB contiguous output DMA on the ACT HWDGE
                       queue (inputs own the SP queue)

The 32x32 block transpose is an involution whose block-nesting (8 | 32)
makes both DCT matmuls use the same I_16 kron D^T stationary weight (one
f32r copy, one bf16 copy) and lands the final result in natural row-major
layout with zero TensorE transposes.
"""

import os

import numpy as np

import concourse.bacc as bacc
import concourse.mybir as mybir
import concourse.tile as tile
from concourse.bass_utils import run_bass_kernel_spmd

N_CORES = 8
B, C, H, W = 64, 3, 512, 512
BLOC = B // N_CORES  # batches per core
P = 128              # SBUF partitions / chunk height
NCH = H // P         # 128-row chunks per image
TOT = BLOC * NCH     # chunks per core
F32 = mybir.dt.float32
F32R = mybir.dt.float32r
BF16 = mybir.dt.bfloat16
GRAY_W = (0.299, 0.587, 0.114)

_NC = None          # cached compiled Bass module
LAST_RUN = None     # BassKernelResults of the most recent run (for test.py)


def _build_bass():
    nc = bacc.Bacc(
        "TRN2",
        target_bir_lowering=False,
        debug=False,
        num_devices=N_CORES,
    )
    x = nc.declare_dram_parameter("x", [BLOC, C, H, W], F32, isOutput=False)
    wts = nc.declare_dram_parameter("wts", [1, P, 2 * P], F32, isOutput=False)
    out = nc.declare_dram_parameter("out", [BLOC, 1, H, W], F32, isOutput=True)

    ga = GRAY_W[0] / GRAY_W[2]
    gb = GRAY_W[1] / GRAY_W[2]
    mult = mybir.AluOpType.mult
    add = mybir.AluOpType.add

    with tile.TileContext(nc) as tc:
        with (
            tc.tile_pool(name="consts", bufs=1) as consts,
            tc.tile_pool(name="xin", bufs=11) as xin,
            tc.tile_pool(name="work", bufs=4) as work,
            tc.tile_pool(name="psum", bufs=3, space="PSUM") as psum_pool,
        ):
            wdf = consts.tile([P, 2 * P], F32, tag="wdf")
            nc.scalar.dma_start(wdf[:], wts[0])
            # stationary weights: f32r for the H-DCT; bf16 plain + bf16
            # row-masked (v<4 zeroed) for the two W-DCT column groups
            wd = consts.tile([P, P], F32R, tag="wd")
            nc.scalar.copy(wd[:], wdf[:, 0:P])
            wdb = consts.tile([P, P], BF16, tag="wdb")
            nc.scalar.copy(wdb[:], wdf[:, 0:P])
            wdm = consts.tile([P, P], BF16, tag="wdm")
            nc.scalar.copy(wdm[:], wdf[:, P:2 * P])

            st = [dict() for _ in range(TOT)]

            def stage_a(k):
                b, hc = divmod(k, NCH)
                # one 768 KB DMA: channels side by side in the free dim
                xt = xin.tile([P, C * W], F32, tag="x")
                xsrc = x[b].rearrange("c (n p) w -> n p c w", p=P)[hc]
                nc.sync.dma_start(
                    xt[:].rearrange("p (c w) -> p c w", w=W), xsrc
                )
                x0 = xt[:, 0 * W:1 * W]
                x1 = xt[:, 1 * W:2 * W]
                x2 = xt[:, 2 * W:3 * W]
                # grayscale split across DVE / ACT / Pool: every cross-engine
                # consumer below runs at lag >= 2, so no in-order engine
                # stream closes a dependency cycle shorter than 2 periods
                g0 = work.tile([P, W], F32, tag="g0")
                nc.vector.scalar_tensor_tensor(g0[:], x0, ga, x2, mult, add)
                gs = work.tile([P, W], F32, tag="gs")
                nc.scalar.mul(gs[:], x1, gb)
                g1 = work.tile([P, W], F32R, tag="g1")
                nc.gpsimd.tensor_tensor(g1[:], gs[:], g0[:], add)
                # H-direction DCT: f32r runs a single full-rate PE pass
                p1 = psum_pool.tile([P, W], F32, tag="p1", bufs=4)
                nc.tensor.matmul(p1[:], wd[:], g1[:], start=True, stop=True)
                st[k]["p1"] = p1

            def stage_b1(k):
                p1 = st[k].pop("p1")
                # PSUM->SBUF move with bf16 cast on ACT (stream transpose
                # cannot change dtypes)
                s1c = work.tile([P, W], BF16, tag="s1c")
                nc.scalar.copy(s1c[:], p1[:])
                st[k]["s1c"] = s1c

            def stage_b2(k):
                s1c = st[k].pop("s1c")
                # 32x32 block transpose on DVE
                s1t = work.tile([P, W], BF16, tag="s1t")
                nc.vector.transpose(s1t[:], s1c[:])
                # W-direction DCT in bf16, split into the two mask column
                # groups: u<4 columns use the v<4-row-zeroed stationary, so
                # the high-pass mask is free
                p2 = psum_pool.tile([P, W], F32, tag="p2", bufs=4)
                s1v = s1t[:].rearrange("p (g u) -> p g u", u=8)
                p2v = p2[:].rearrange("p (g u) -> p g u", u=8)
                nc.tensor.matmul(
                    p2v[:, :, 0:4], wdm[:], s1v[:, :, 0:4],
                    start=True, stop=True,
                )
                nc.tensor.matmul(
                    p2v[:, :, 4:8], wdb[:], s1v[:, :, 4:8],
                    start=True, stop=True,
                )
                st[k]["p2"] = p2

            def stage_c(k):
                p2 = st[k].pop("p2")
                # plain PSUM->SBUF move on ACT (mask already folded into
                # MM2's stationary), then block transpose back to natural
                # layout on DVE from SBUF (PSUM-source transposes schedule
                # much worse)
                s2 = work.tile([P, W], F32, tag="s2")
                nc.scalar.copy(s2[:], p2[:])
                s2t = work.tile([P, W], F32, tag="s2t", bufs=8)
                nc.vector.transpose(s2t[:], s2[:])
                st[k]["s2t"] = s2t

            def stage_d(k):
                b, hc = divmod(k, NCH)
                hs = hc * P
                s2t = st[k].pop("s2t")
                # outputs ride the SP queue at deep lag (11): the in-order
                # SP sequencer caps input prefetch at the output lag, but 11
                # matches the xin depth, and the compute period is below the
                # DMA period so the wait is always satisfied; this keeps the
                # ACT sequencer free of descriptor generation
                nc.sync.dma_start(out[b, 0, hs:hs + P, :], s2t[:])

            LAG = 11
            for it in range(TOT + LAG):
                if it < TOT:
                    stage_a(it)
                if 1 <= it < TOT + 1:
                    stage_b1(it - 1)
                    stage_b2(it - 1)
                if 2 <= it < TOT + 2:
                    stage_c(it - 2)
                if LAG <= it:
                    stage_d(it - LAG)
    nc.compile()
    return nc


def _host_constants(dct_matrix, mask):
    D = np.asarray(dct_matrix, dtype=np.float32)
    M = np.asarray(mask, dtype=np.float32)
    dctT = np.kron(np.eye(P // 8, dtype=np.float32), D.T).astype(np.float32)
    # fold the trailing grayscale scale (GRAY_W[2]) into the (shared) DCT
    # weight as sqrt(c): both matmuls apply it, so the chain gains c total.
    base = (np.sqrt(np.float32(GRAY_W[2])) * dctT).astype(np.float32)
    # masked variant for the u<4 moving-column group of MM2: zero the
    # stationary's free columns i with i%8 < 4 (those produce the v<4
    # output partitions, which the high-pass mask zeroes when u<4).
    colmask = (np.arange(P) % 8 >= 4).astype(np.float32)[None, :]
    # M[u<4, v<4] == 0 in the reference; sanity-anchor the derivation
    assert M[0, 0] == 0.0 and M[0, 4] == 1.0 and M[4, 0] == 1.0
    wts = np.concatenate([base, base * colmask], axis=1)[None]
    return wts


def kernel(x, dct_matrix, mask):
    global _NC, LAST_RUN
    x = np.ascontiguousarray(np.asarray(x, dtype=np.float32))
    assert x.shape == (B, C, H, W)
    wts = _host_constants(dct_matrix, mask)

    if _NC is None:
        _NC = _build_bass()

    in_maps = [
        {"x": np.ascontiguousarray(x[i * BLOC:(i + 1) * BLOC]), "wts": wts}
        for i in range(N_CORES)
    ]
    trace = bool(int(os.environ.get("DCT_TRACE", "0")))
    LAST_RUN = run_bass_kernel_spmd(
        _NC, in_maps, list(range(N_CORES)), trace=trace,
    )
    out = np.concatenate([LAST_RUN.results[i]["out"] for i in range(N_CORES)], axis=0)
    return out


# revision 35
# speedup vs baseline: 1.0036x; 1.0036x over previous
"""Trainium2 Bass kernel for blocked-DCT high-frequency extractor.

Computes, for x (64, 3, 512, 512) f32:
  gray = 0.299*R + 0.587*G + 0.114*B                     (B,1,H,W)
  per 8x8 block:  Y = mask * (D @ block @ D.T)           (2D DCT + high-pass)
  output (64, 1, 512, 512) f32

Strategy: pure data parallel over batch (8 batches/core on 8 cores). The
kernel is HBM-bound: 24 MiB in + 8 MiB out per core => ~94 us floor at the
~358 GB/s per-core HBM limit. All compute engines are kept below the DMA
stream rate (~2.74 us per 128-row chunk) and the per-chunk work is
SOFTWARE-PIPELINED across three stages so the in-order engine streams never
serialize on the cross-engine dependency chain:

  stage A (chunk k):   in-DMA -> gray (DVE stt / ACT mul / GpSimd add,
                       rounded to f32r) -> H-DCT matmul (f32r, 1 cyc/row)
  stage B (chunk k-1): ACT PSUM->SBUF cast to bf16 -> DVE 32x32 block
                       transpose (bf16) -> W-DCT matmul (bf16)
  stage C (chunk k-2): high-pass mask on ACT (two strided PSUM->SBUF
                       copies; u<4 columns scaled by a per-partition 0/1
                       vector) -> DVE block transpose back (f32)
  stage D (chunk k-3): 256 KB contiguous output DMA on the ACT HWDGE
                       queue (inputs own the SP queue)

The 32x32 block transpose is an involution whose block-nesting (8 | 32)
makes both DCT matmuls use the same I_16 kron D^T stationary weight (one
f32r copy, one bf16 copy) and lands the final result in natural row-major
layout with zero TensorE transposes.
"""

import os

import numpy as np

import concourse.bacc as bacc
import concourse.mybir as mybir
import concourse.tile as tile
from concourse.bass_utils import run_bass_kernel_spmd

N_CORES = 8
B, C, H, W = 64, 3, 512, 512
BLOC = B // N_CORES  # batches per core
P = 128              # SBUF partitions / chunk height
NCH = H // P         # 128-row chunks per image
TOT = BLOC * NCH     # chunks per core
F32 = mybir.dt.float32
F32R = mybir.dt.float32r
BF16 = mybir.dt.bfloat16
GRAY_W = (0.299, 0.587, 0.114)

_NC = None          # cached compiled Bass module
LAST_RUN = None     # BassKernelResults of the most recent run (for test.py)


def _build_bass():
    nc = bacc.Bacc(
        "TRN2",
        target_bir_lowering=False,
        debug=False,
        num_devices=N_CORES,
    )
    x = nc.declare_dram_parameter("x", [BLOC, C, H, W], F32, isOutput=False)
    wts = nc.declare_dram_parameter("wts", [1, P, 2 * P], F32, isOutput=False)
    out = nc.declare_dram_parameter("out", [BLOC, 1, H, W], F32, isOutput=True)

    ga = GRAY_W[0] / GRAY_W[2]
    gb = GRAY_W[1] / GRAY_W[2]
    mult = mybir.AluOpType.mult
    add = mybir.AluOpType.add

    with tile.TileContext(nc) as tc:
        with (
            tc.tile_pool(name="consts", bufs=1) as consts,
            tc.tile_pool(name="xin", bufs=11) as xin,
            tc.tile_pool(name="work", bufs=4) as work,
            tc.tile_pool(name="psum", bufs=3, space="PSUM") as psum_pool,
        ):
            wdf = consts.tile([P, 2 * P], F32, tag="wdf")
            nc.scalar.dma_start(wdf[:], wts[0])
            # stationary weights: f32r for the H-DCT; bf16 plain + bf16
            # row-masked (v<4 zeroed) for the two W-DCT column groups
            wd = consts.tile([P, P], F32R, tag="wd")
            nc.scalar.copy(wd[:], wdf[:, 0:P])
            wdb = consts.tile([P, P], BF16, tag="wdb")
            nc.scalar.copy(wdb[:], wdf[:, 0:P])
            wdm = consts.tile([P, P], BF16, tag="wdm")
            nc.scalar.copy(wdm[:], wdf[:, P:2 * P])

            st = [dict() for _ in range(TOT)]

            def stage_a(k):
                b, hc = divmod(k, NCH)
                # one 768 KB DMA: channels side by side in the free dim
                xt = xin.tile([P, C * W], F32, tag="x")
                xsrc = x[b].rearrange("c (n p) w -> n p c w", p=P)[hc]
                nc.sync.dma_start(
                    xt[:].rearrange("p (c w) -> p c w", w=W), xsrc
                )
                x0 = xt[:, 0 * W:1 * W]
                x1 = xt[:, 1 * W:2 * W]
                x2 = xt[:, 2 * W:3 * W]
                # grayscale split across DVE / ACT / Pool: every cross-engine
                # consumer below runs at lag >= 2, so no in-order engine
                # stream closes a dependency cycle shorter than 2 periods
                g0 = work.tile([P, W], F32, tag="g0")
                nc.vector.scalar_tensor_tensor(g0[:], x0, ga, x2, mult, add)
                gs = work.tile([P, W], F32, tag="gs")
                nc.scalar.mul(gs[:], x1, gb)
                g1 = work.tile([P, W], F32R, tag="g1")
                nc.gpsimd.tensor_tensor(g1[:], gs[:], g0[:], add)
                # H-direction DCT: f32r runs a single full-rate PE pass
                p1 = psum_pool.tile([P, W], F32, tag="p1", bufs=4)
                nc.tensor.matmul(p1[:], wd[:], g1[:], start=True, stop=True)
                st[k]["p1"] = p1

            def stage_b1(k):
                p1 = st[k].pop("p1")
                # PSUM->SBUF move with bf16 cast on ACT (stream transpose
                # cannot change dtypes)
                s1c = work.tile([P, W], BF16, tag="s1c")
                nc.scalar.copy(s1c[:], p1[:])
                st[k]["s1c"] = s1c

            def stage_b2(k):
                s1c = st[k].pop("s1c")
                # 32x32 block transpose on DVE
                s1t = work.tile([P, W], BF16, tag="s1t")
                nc.vector.transpose(s1t[:], s1c[:])
                # W-direction DCT in bf16, split into the two mask column
                # groups: u<4 columns use the v<4-row-zeroed stationary, so
                # the high-pass mask is free
                p2 = psum_pool.tile([P, W], F32, tag="p2", bufs=4)
                s1v = s1t[:].rearrange("p (g u) -> p g u", u=8)
                p2v = p2[:].rearrange("p (g u) -> p g u", u=8)
                nc.tensor.matmul(
                    p2v[:, :, 0:4], wdm[:], s1v[:, :, 0:4],
                    start=True, stop=True,
                )
                nc.tensor.matmul(
                    p2v[:, :, 4:8], wdb[:], s1v[:, :, 4:8],
                    start=True, stop=True,
                )
                st[k]["p2"] = p2

            def stage_c(k):
                p2 = st[k].pop("p2")
                # plain PSUM->SBUF move on ACT (mask already folded into
                # MM2's stationary), then block transpose back to natural
                # layout on DVE from SBUF (PSUM-source transposes schedule
                # much worse)
                s2 = work.tile([P, W], F32, tag="s2")
                nc.scalar.copy(s2[:], p2[:])
                s2t = work.tile([P, W], F32, tag="s2t", bufs=13)
                nc.vector.transpose(s2t[:], s2[:])
                st[k]["s2t"] = s2t

            def stage_d(k):
                b, hc = divmod(k, NCH)
                hs = hc * P
                s2t = st[k].pop("s2t")
                # outputs ride the SP queue at deep lag (11): the in-order
                # SP sequencer caps input prefetch at the output lag, but 11
                # matches the xin depth, and the compute period is below the
                # DMA period so the wait is always satisfied; this keeps the
                # ACT sequencer free of descriptor generation
                nc.sync.dma_start(out[b, 0, hs:hs + P, :], s2t[:])

            LAG = 11
            for it in range(TOT + LAG):
                if it < TOT:
                    stage_a(it)
                if 1 <= it < TOT + 1:
                    stage_b1(it - 1)
                    stage_b2(it - 1)
                if 2 <= it < TOT + 2:
                    stage_c(it - 2)
                if LAG <= it:
                    stage_d(it - LAG)
    nc.compile()
    return nc


def _host_constants(dct_matrix, mask):
    D = np.asarray(dct_matrix, dtype=np.float32)
    M = np.asarray(mask, dtype=np.float32)
    dctT = np.kron(np.eye(P // 8, dtype=np.float32), D.T).astype(np.float32)
    # fold the trailing grayscale scale (GRAY_W[2]) into the (shared) DCT
    # weight as sqrt(c): both matmuls apply it, so the chain gains c total.
    base = (np.sqrt(np.float32(GRAY_W[2])) * dctT).astype(np.float32)
    # masked variant for the u<4 moving-column group of MM2: zero the
    # stationary's free columns i with i%8 < 4 (those produce the v<4
    # output partitions, which the high-pass mask zeroes when u<4).
    colmask = (np.arange(P) % 8 >= 4).astype(np.float32)[None, :]
    # M[u<4, v<4] == 0 in the reference; sanity-anchor the derivation
    assert M[0, 0] == 0.0 and M[0, 4] == 1.0 and M[4, 0] == 1.0
    wts = np.concatenate([base, base * colmask], axis=1)[None]
    return wts


def kernel(x, dct_matrix, mask):
    global _NC, LAST_RUN
    x = np.ascontiguousarray(np.asarray(x, dtype=np.float32))
    assert x.shape == (B, C, H, W)
    wts = _host_constants(dct_matrix, mask)

    if _NC is None:
        _NC = _build_bass()

    in_maps = [
        {"x": np.ascontiguousarray(x[i * BLOC:(i + 1) * BLOC]), "wts": wts}
        for i in range(N_CORES)
    ]
    trace = bool(int(os.environ.get("DCT_TRACE", "0")))
    LAST_RUN = run_bass_kernel_spmd(
        _NC, in_maps, list(range(N_CORES)), trace=trace,
    )
    out = np.concatenate([LAST_RUN.results[i]["out"] for i in range(N_CORES)], axis=0)
    return out


# revision 36
# speedup vs baseline: 1.2156x; 1.2112x over previous
"""Trainium2 Bass kernel for blocked-DCT high-frequency extractor.

Computes, for x (64, 3, 512, 512) f32:
  gray = 0.299*R + 0.587*G + 0.114*B                     (B,1,H,W)
  per 8x8 block:  Y = mask * (D @ block @ D.T)           (2D DCT + high-pass)
  output (64, 1, 512, 512) f32

Strategy: pure data parallel over batch (8 batches/core on 8 cores). The
kernel is HBM-bound: 24 MiB in + 8 MiB out per core => ~94 us floor at the
~358 GB/s per-core HBM limit. All compute engines are kept below the DMA
stream rate (~2.74 us per 128-row chunk) and the per-chunk work is
SOFTWARE-PIPELINED across three stages so the in-order engine streams never
serialize on the cross-engine dependency chain:

  stage A (chunk k):   in-DMA -> gray (DVE stt / ACT mul / GpSimd add,
                       rounded to f32r) -> H-DCT matmul (f32r, 1 cyc/row)
  stage B (chunk k-1): ACT PSUM->SBUF cast to bf16 -> DVE 32x32 block
                       transpose (bf16) -> W-DCT matmul (bf16)
  stage C (chunk k-2): high-pass mask on ACT (two strided PSUM->SBUF
                       copies; u<4 columns scaled by a per-partition 0/1
                       vector) -> DVE block transpose back (f32)
  stage D (chunk k-3): 256 KB contiguous output DMA on the ACT HWDGE
                       queue (inputs own the SP queue)

The 32x32 block transpose is an involution whose block-nesting (8 | 32)
makes both DCT matmuls use the same I_16 kron D^T stationary weight (one
f32r copy, one bf16 copy) and lands the final result in natural row-major
layout with zero TensorE transposes.
"""

import os

import numpy as np

import concourse.bacc as bacc
import concourse.mybir as mybir
import concourse.tile as tile
from concourse.bass_utils import run_bass_kernel_spmd

N_CORES = 8
B, C, H, W = 64, 3, 512, 512
BLOC = B // N_CORES  # batches per core
P = 128              # SBUF partitions / chunk height
NCH = H // P         # 128-row chunks per image
TOT = BLOC * NCH     # chunks per core
F32 = mybir.dt.float32
F32R = mybir.dt.float32r
BF16 = mybir.dt.bfloat16
GRAY_W = (0.299, 0.587, 0.114)

_NC = None          # cached compiled Bass module
LAST_RUN = None     # BassKernelResults of the most recent run (for test.py)


def _build_bass():
    nc = bacc.Bacc(
        "TRN2",
        target_bir_lowering=False,
        debug=False,
        num_devices=N_CORES,
    )
    x = nc.declare_dram_parameter("x", [BLOC, C, H, W], F32, isOutput=False)
    wts = nc.declare_dram_parameter("wts", [1, P, 2 * P], F32, isOutput=False)
    out = nc.declare_dram_parameter("out", [BLOC, 1, H, W], F32, isOutput=True)

    ga = GRAY_W[0] / GRAY_W[2]
    gb = GRAY_W[1] / GRAY_W[2]
    mult = mybir.AluOpType.mult
    add = mybir.AluOpType.add

    with tile.TileContext(nc) as tc:
        with (
            tc.tile_pool(name="consts", bufs=1) as consts,
            tc.tile_pool(name="xin", bufs=11) as xin,
            tc.tile_pool(name="work", bufs=4) as work,
            tc.tile_pool(name="psum", bufs=3, space="PSUM") as psum_pool,
        ):
            wdf = consts.tile([P, 2 * P], F32, tag="wdf")
            nc.scalar.dma_start(wdf[:], wts[0])
            # stationary weights: f32r for the H-DCT; bf16 plain + bf16
            # row-masked (v<4 zeroed) for the two W-DCT column groups
            wd = consts.tile([P, P], F32R, tag="wd")
            nc.scalar.copy(wd[:], wdf[:, 0:P])
            wdb = consts.tile([P, P], BF16, tag="wdb")
            nc.scalar.copy(wdb[:], wdf[:, 0:P])
            wdm = consts.tile([P, P], BF16, tag="wdm")
            nc.scalar.copy(wdm[:], wdf[:, P:2 * P])

            st = [dict() for _ in range(TOT)]

            def stage_a(k):
                b, hc = divmod(k, NCH)
                # one 768 KB DMA: channels side by side in the free dim
                xt = xin.tile([P, C * W], F32, tag="x")
                xsrc = x[b].rearrange("c (n p) w -> n p c w", p=P)[hc]
                nc.sync.dma_start(
                    xt[:].rearrange("p (c w) -> p c w", w=W), xsrc
                )
                x0 = xt[:, 0 * W:1 * W]
                x1 = xt[:, 1 * W:2 * W]
                x2 = xt[:, 2 * W:3 * W]
                # grayscale split across DVE / ACT / Pool: every cross-engine
                # consumer below runs at lag >= 2, so no in-order engine
                # stream closes a dependency cycle shorter than 2 periods
                g0 = work.tile([P, W], F32, tag="g0")
                nc.vector.scalar_tensor_tensor(g0[:], x0, ga, x2, mult, add)
                gs = work.tile([P, W], F32, tag="gs")
                nc.scalar.mul(gs[:], x1, gb)
                g1 = work.tile([P, W], F32R, tag="g1")
                nc.gpsimd.tensor_tensor(g1[:], gs[:], g0[:], add)
                # H-direction DCT: f32r runs a single full-rate PE pass
                p1 = psum_pool.tile([P, W], F32, tag="p1", bufs=4)
                nc.tensor.matmul(p1[:], wd[:], g1[:], start=True, stop=True)
                st[k]["p1"] = p1

            def stage_b1(k):
                p1 = st[k].pop("p1")
                # PSUM->SBUF move with bf16 cast on ACT (stream transpose
                # cannot change dtypes)
                s1c = work.tile([P, W], BF16, tag="s1c")
                nc.scalar.copy(s1c[:], p1[:])
                st[k]["s1c"] = s1c

            def stage_b2(k):
                s1c = st[k].pop("s1c")
                # 32x32 block transpose on DVE
                s1t = work.tile([P, W], BF16, tag="s1t")
                nc.vector.transpose(s1t[:], s1c[:])
                # W-direction DCT in bf16, split into the two mask column
                # groups: u<4 columns use the v<4-row-zeroed stationary, so
                # the high-pass mask is free
                p2 = psum_pool.tile([P, W], F32, tag="p2", bufs=4)
                s1v = s1t[:].rearrange("p (g u) -> p g u", u=8)
                p2v = p2[:].rearrange("p (g u) -> p g u", u=8)
                nc.tensor.matmul(
                    p2v[:, :, 0:4], wdm[:], s1v[:, :, 0:4],
                    start=True, stop=True,
                )
                nc.tensor.matmul(
                    p2v[:, :, 4:8], wdb[:], s1v[:, :, 4:8],
                    start=True, stop=True,
                )
                st[k]["p2"] = p2

            def stage_c(k):
                p2 = st[k].pop("p2")
                # plain PSUM->SBUF move on ACT (mask already folded into
                # MM2's stationary), then block transpose back to natural
                # layout on DVE from SBUF (PSUM-source transposes schedule
                # much worse)
                s2 = work.tile([P, W], F32, tag="s2")
                nc.scalar.copy(s2[:], p2[:])
                s2t = work.tile([P, W], F32, tag="s2t", bufs=8)
                nc.vector.transpose(s2t[:], s2[:])
                st[k]["s2t"] = s2t

            def stage_d(k):
                b, hc = divmod(k, NCH)
                hs = hc * P
                s2t = st[k].pop("s2t")
                # outputs ride the SWDGE (GpSimd) ring: HWDGE rings execute
                # their DMAs in strict FIFO order, so outputs sharing the SP
                # ring would act as barriers inside the input stream, and the
                # ACT sequencer serializes descriptor generation with its
                # engine ops
                nc.gpsimd.dma_start(out[b, 0, hs:hs + P, :], s2t[:])

            LAG = 3
            for it in range(TOT + LAG):
                if it < TOT:
                    stage_a(it)
                if 1 <= it < TOT + 1:
                    stage_b1(it - 1)
                    stage_b2(it - 1)
                if 2 <= it < TOT + 2:
                    stage_c(it - 2)
                if LAG <= it:
                    stage_d(it - LAG)
    nc.compile()
    return nc


def _host_constants(dct_matrix, mask):
    D = np.asarray(dct_matrix, dtype=np.float32)
    M = np.asarray(mask, dtype=np.float32)
    dctT = np.kron(np.eye(P // 8, dtype=np.float32), D.T).astype(np.float32)
    # fold the trailing grayscale scale (GRAY_W[2]) into the (shared) DCT
    # weight as sqrt(c): both matmuls apply it, so the chain gains c total.
    base = (np.sqrt(np.float32(GRAY_W[2])) * dctT).astype(np.float32)
    # masked variant for the u<4 moving-column group of MM2: zero the
    # stationary's free columns i with i%8 < 4 (those produce the v<4
    # output partitions, which the high-pass mask zeroes when u<4).
    colmask = (np.arange(P) % 8 >= 4).astype(np.float32)[None, :]
    # M[u<4, v<4] == 0 in the reference; sanity-anchor the derivation
    assert M[0, 0] == 0.0 and M[0, 4] == 1.0 and M[4, 0] == 1.0
    wts = np.concatenate([base, base * colmask], axis=1)[None]
    return wts


def kernel(x, dct_matrix, mask):
    global _NC, LAST_RUN
    x = np.ascontiguousarray(np.asarray(x, dtype=np.float32))
    assert x.shape == (B, C, H, W)
    wts = _host_constants(dct_matrix, mask)

    if _NC is None:
        _NC = _build_bass()

    in_maps = [
        {"x": np.ascontiguousarray(x[i * BLOC:(i + 1) * BLOC]), "wts": wts}
        for i in range(N_CORES)
    ]
    trace = bool(int(os.environ.get("DCT_TRACE", "0")))
    LAST_RUN = run_bass_kernel_spmd(
        _NC, in_maps, list(range(N_CORES)), trace=trace,
    )
    out = np.concatenate([LAST_RUN.results[i]["out"] for i in range(N_CORES)], axis=0)
    return out


# revision 37
# speedup vs baseline: 1.4185x; 1.1670x over previous
"""Trainium2 Bass kernel for blocked-DCT high-frequency extractor.

Computes, for x (64, 3, 512, 512) f32:
  gray = 0.299*R + 0.587*G + 0.114*B                     (B,1,H,W)
  per 8x8 block:  Y = mask * (D @ block @ D.T)           (2D DCT + high-pass)
  output (64, 1, 512, 512) f32

Strategy: pure data parallel over batch (8 batches/core on 8 cores). The
kernel is HBM-bound: 24 MiB in + 8 MiB out per core => ~94 us floor at the
~358 GB/s per-core HBM limit. All compute engines are kept below the DMA
stream rate (~2.74 us per 128-row chunk) and the per-chunk work is
SOFTWARE-PIPELINED across three stages so the in-order engine streams never
serialize on the cross-engine dependency chain:

  stage A (chunk k):   in-DMA -> gray (DVE stt / ACT mul / GpSimd add,
                       rounded to f32r) -> H-DCT matmul (f32r, 1 cyc/row)
  stage B (chunk k-1): ACT PSUM->SBUF cast to bf16 -> DVE 32x32 block
                       transpose (bf16) -> W-DCT matmul (bf16)
  stage C (chunk k-2): high-pass mask on ACT (two strided PSUM->SBUF
                       copies; u<4 columns scaled by a per-partition 0/1
                       vector) -> DVE block transpose back (f32)
  stage D (chunk k-3): 256 KB contiguous output DMA on the ACT HWDGE
                       queue (inputs own the SP queue)

The 32x32 block transpose is an involution whose block-nesting (8 | 32)
makes both DCT matmuls use the same I_16 kron D^T stationary weight (one
f32r copy, one bf16 copy) and lands the final result in natural row-major
layout with zero TensorE transposes.
"""

import os

import numpy as np

import concourse.bacc as bacc
import concourse.mybir as mybir
import concourse.tile as tile
from concourse.bass_utils import run_bass_kernel_spmd

N_CORES = 8
B, C, H, W = 64, 3, 512, 512
BLOC = B // N_CORES  # batches per core
P = 128              # SBUF partitions / chunk height
NCH = H // P         # 128-row chunks per image
TOT = BLOC * NCH     # chunks per core
F32 = mybir.dt.float32
F32R = mybir.dt.float32r
BF16 = mybir.dt.bfloat16
GRAY_W = (0.299, 0.587, 0.114)

_NC = None          # cached compiled Bass module
LAST_RUN = None     # BassKernelResults of the most recent run (for test.py)


def _build_bass():
    nc = bacc.Bacc(
        "TRN2",
        target_bir_lowering=False,
        debug=False,
        num_devices=N_CORES,
    )
    x = nc.declare_dram_parameter("x", [BLOC, C, H, W], F32, isOutput=False)
    wts = nc.declare_dram_parameter("wts", [1, P, 2 * P], F32, isOutput=False)
    out = nc.declare_dram_parameter("out", [BLOC, 1, H, W], F32, isOutput=True)

    ga = GRAY_W[0] / GRAY_W[2]
    gb = GRAY_W[1] / GRAY_W[2]
    mult = mybir.AluOpType.mult
    add = mybir.AluOpType.add

    with tile.TileContext(nc) as tc:
        with (
            tc.tile_pool(name="consts", bufs=1) as consts,
            tc.tile_pool(name="xin", bufs=11) as xin,
            tc.tile_pool(name="work", bufs=4) as work,
            tc.tile_pool(name="psum", bufs=3, space="PSUM") as psum_pool,
        ):
            wdf = consts.tile([P, 2 * P], F32, tag="wdf")
            nc.scalar.dma_start(wdf[:], wts[0])
            # stationary weights: f32r for the H-DCT; bf16 plain + bf16
            # row-masked (v<4 zeroed) for the two W-DCT column groups
            wd = consts.tile([P, P], F32R, tag="wd")
            nc.scalar.copy(wd[:], wdf[:, 0:P])
            wdb = consts.tile([P, P], BF16, tag="wdb")
            nc.scalar.copy(wdb[:], wdf[:, 0:P])
            wdm = consts.tile([P, P], BF16, tag="wdm")
            nc.scalar.copy(wdm[:], wdf[:, P:2 * P])

            st = [dict() for _ in range(TOT)]

            def stage_a(k):
                b, hc = divmod(k, NCH)
                # one 768 KB DMA: channels side by side in the free dim
                xt = xin.tile([P, C * W], F32, tag="x")
                xsrc = x[b].rearrange("c (n p) w -> n p c w", p=P)[hc]
                nc.sync.dma_start(
                    xt[:].rearrange("p (c w) -> p c w", w=W), xsrc
                )
                x0 = xt[:, 0 * W:1 * W]
                x1 = xt[:, 1 * W:2 * W]
                x2 = xt[:, 2 * W:3 * W]
                # grayscale split across DVE / ACT / Pool: every cross-engine
                # consumer below runs at lag >= 2, so no in-order engine
                # stream closes a dependency cycle shorter than 2 periods
                g0 = work.tile([P, W], F32, tag="g0")
                nc.vector.scalar_tensor_tensor(g0[:], x0, ga, x2, mult, add)
                gs = work.tile([P, W], F32, tag="gs")
                nc.scalar.mul(gs[:], x1, gb)
                g1 = work.tile([P, W], F32R, tag="g1")
                nc.gpsimd.tensor_tensor(g1[:], gs[:], g0[:], add)
                # H-direction DCT: f32r runs a single full-rate PE pass
                p1 = psum_pool.tile([P, W], F32, tag="p1", bufs=4)
                nc.tensor.matmul(p1[:], wd[:], g1[:], start=True, stop=True)
                st[k]["p1"] = p1

            def stage_b1(k):
                p1 = st[k].pop("p1")
                # PSUM->SBUF move with bf16 cast on ACT (stream transpose
                # cannot change dtypes)
                s1c = work.tile([P, W], BF16, tag="s1c")
                nc.scalar.copy(s1c[:], p1[:])
                st[k]["s1c"] = s1c

            def stage_b2(k):
                s1c = st[k].pop("s1c")
                # 32x32 block transpose on DVE
                s1t = work.tile([P, W], BF16, tag="s1t")
                nc.vector.transpose(s1t[:], s1c[:])
                # W-direction DCT in bf16, split into the two mask column
                # groups: u<4 columns use the v<4-row-zeroed stationary, so
                # the high-pass mask is free
                p2 = psum_pool.tile([P, W], F32, tag="p2", bufs=4)
                s1v = s1t[:].rearrange("p (g u) -> p g u", u=8)
                p2v = p2[:].rearrange("p (g u) -> p g u", u=8)
                nc.tensor.matmul(
                    p2v[:, :, 0:4], wdm[:], s1v[:, :, 0:4],
                    start=True, stop=True,
                )
                nc.tensor.matmul(
                    p2v[:, :, 4:8], wdb[:], s1v[:, :, 4:8],
                    start=True, stop=True,
                )
                st[k]["p2"] = p2

            def stage_c(k):
                p2 = st[k].pop("p2")
                # plain PSUM->SBUF move on ACT (mask already folded into
                # MM2's stationary), then block transpose back to natural
                # layout on DVE from SBUF (PSUM-source transposes schedule
                # much worse)
                s2 = work.tile([P, W], F32, tag="s2")
                nc.scalar.copy(s2[:], p2[:])
                s2t = work.tile([P, W], F32, tag="s2t", bufs=8)
                nc.vector.transpose(s2t[:], s2[:])
                st[k]["s2t"] = s2t

            def stage_d(k):
                b, hc = divmod(k, NCH)
                hs = hc * P
                s2t = st[k].pop("s2t")
                # outputs ride the ACT HWDGE ring: HWDGE rings execute their
                # DMAs in strict FIFO order, so outputs sharing the SP ring
                # would act as barriers inside the input stream; SWDGE
                # (GpSimd) descriptor generation is too slow
                nc.scalar.dma_start(out[b, 0, hs:hs + P, :], s2t[:])

            LAG = 3
            for it in range(TOT + LAG):
                if it < TOT:
                    stage_a(it)
                if 1 <= it < TOT + 1:
                    stage_b1(it - 1)
                    stage_b2(it - 1)
                if 2 <= it < TOT + 2:
                    stage_c(it - 2)
                if LAG <= it:
                    stage_d(it - LAG)
    nc.compile()
    return nc


def _host_constants(dct_matrix, mask):
    D = np.asarray(dct_matrix, dtype=np.float32)
    M = np.asarray(mask, dtype=np.float32)
    dctT = np.kron(np.eye(P // 8, dtype=np.float32), D.T).astype(np.float32)
    # fold the trailing grayscale scale (GRAY_W[2]) into the (shared) DCT
    # weight as sqrt(c): both matmuls apply it, so the chain gains c total.
    base = (np.sqrt(np.float32(GRAY_W[2])) * dctT).astype(np.float32)
    # masked variant for the u<4 moving-column group of MM2: zero the
    # stationary's free columns i with i%8 < 4 (those produce the v<4
    # output partitions, which the high-pass mask zeroes when u<4).
    colmask = (np.arange(P) % 8 >= 4).astype(np.float32)[None, :]
    # M[u<4, v<4] == 0 in the reference; sanity-anchor the derivation
    assert M[0, 0] == 0.0 and M[0, 4] == 1.0 and M[4, 0] == 1.0
    wts = np.concatenate([base, base * colmask], axis=1)[None]
    return wts


def kernel(x, dct_matrix, mask):
    global _NC, LAST_RUN
    x = np.ascontiguousarray(np.asarray(x, dtype=np.float32))
    assert x.shape == (B, C, H, W)
    wts = _host_constants(dct_matrix, mask)

    if _NC is None:
        _NC = _build_bass()

    in_maps = [
        {"x": np.ascontiguousarray(x[i * BLOC:(i + 1) * BLOC]), "wts": wts}
        for i in range(N_CORES)
    ]
    trace = bool(int(os.environ.get("DCT_TRACE", "0")))
    LAST_RUN = run_bass_kernel_spmd(
        _NC, in_maps, list(range(N_CORES)), trace=trace,
    )
    out = np.concatenate([LAST_RUN.results[i]["out"] for i in range(N_CORES)], axis=0)
    return out
